# revision 35
# baseline (speedup 1.0000x reference)
"""Trainium2 Bass kernel for BinaryTreeLatentVariable inside algorithm.

Math (per level, bottom-up over a complete binary tree in heap order):
    new[pp, n] = p[pp, n] + logsumexp_{i,j}( trans[pp,i,j] + l[i,n] + r[j,n] )

CP factorization: exp(trans)[pp,i,j] ~= sum_r U[pp,r] V[i,r] W[j,r] (rank-32
ALS fit; the output is dominated by the accumulated log-partition offset, so
the ~7% tensor fit error contributes only ~1e-4 relative error):

    S[pp, n] = sum_r U[pp,r] * (V^T Fl)[r,n] * (W^T Fr)[r,n]

with F the child values in EXP space.  Levels alternate:
  FAST (8, 6, 4, 2): F_d = exp(sw_abs) * S_d  -- exp-space, fully absolute,
      no ln / normalization / z bookkeeping (dense multiplies).
  FULL (7, 5, 3, 1): t = ln(S * 2^-48) (the 2^-48 keeps t in bf16 range; the
      shift is repaid as a host constant), per-tree z capture of t[0] and of
      the emission state-0 row, then y = (t - t0) + sw_norm re-normalizes.
Absolute values drift by only ~2 levels of accumulation before a FULL level
renormalizes, so exp-space magnitudes stay < e^30 (bf16 max e^88).

Emission: h cast host-side to fp8e4 (halving HBM traffic vs bf16), two
DoubleRow matmuls (K=256 each) producing BOTH a normalized-weight block
(rows 0..19: W_i - W_0, row0 zeroed via the ACT scale trick) and a raw-weight
block (rows 32..51) in one pass -- output partitions are free.  FULL levels
consume the normalized block (Identity), FAST levels and leaves the raw block
(Exp, directly exp-space).

Layout: columns level-major (leaves first) so deep levels overlap the h DMA
tail; within a level BIT-REVERSED node order with the 8 trees innermost, so
left children always occupy the first half of a level block and every
pair-deinterleave is a dense block copy; per-level buffers deinterleaved
(left child partitions 0..19, right child 32..51) so one dense op feeds both
CP factor matmuls.

Sharding: 8 trees per core across 8 cores (no cross-core communication).
"""

import ml_dtypes
import numpy as np

import concourse.bacc as bacc
import concourse.bass as bass
from concourse import mybir, tile
from concourse.bass_utils import run_bass_kernel_spmd

F32 = mybir.dt.float32
BF16 = mybir.dt.bfloat16
FP8 = mybir.dt.float8e4
NP_BF16 = ml_dtypes.bfloat16
NP_FP8 = ml_dtypes.float8_e4m3

B = 64
N_NODES = 1023
D = 512
L = 5
C = 4
LC = L * C          # 20
NCORES = 8
TPC = B // NCORES   # trees per core = 8
DEPTH = 9           # leaves are level 9; internal levels 8..0
R = 32              # CP rank

FAST = {8, 6, 4, 2}
SLN = 2.0 ** -48
SHIFT = 48 * float(np.log(2.0))

LEVEL_ORDER = list(range(DEPTH, -1, -1))  # 9, 8, ..., 0
OFFS = {}
_off = 0
for _d in LEVEL_ORDER:
    OFFS[_d] = _off
    _off += TPC * (1 << _d)
NCOL = _off                      # 8184
NCOLP = 8192                     # padded
NLEAFC = TPC * (1 << DEPTH)      # 4096 leaf columns
NCOLI = NCOLP - NLEAFC           # 4096 internal columns (incl 8 pad)
OFFSI = {d: OFFS[d] - NLEAFC for d in range(DEPTH)}

COLTILE = 512
DMATILE = 2048
NDMAT = NCOLP // DMATILE         # 8
EROW = 32                        # right-sibling partition base


def _cp_fit(trans):
    """Rank-R ALS CP fit of exp(trans) as [pp,(lL,lc),(rL,rc)]."""
    T = np.exp(trans.astype(np.float64).transpose(0, 3, 1, 4, 2, 5)
               .reshape(LC, LC, LC))
    rng = np.random.default_rng(0)
    U = rng.uniform(0.5, 1.5, (LC, R))
    V = rng.uniform(0.5, 1.5, (LC, R))
    W = rng.uniform(0.5, 1.5, (LC, R))
    T1 = T.reshape(LC, -1)
    T2 = T.transpose(1, 0, 2).reshape(LC, -1)
    T3 = T.transpose(2, 0, 1).reshape(LC, -1)

    def khatri(A, Bm):
        return (A[:, None, :] * Bm[None, :, :]).reshape(-1, A.shape[1])

    eye = 1e-10 * np.eye(R)
    for _ in range(200):
        for mode in range(3):
            if mode == 0:
                K, M = khatri(V, W), T1
            elif mode == 1:
                K, M = khatri(U, W), T2
            else:
                K, M = khatri(U, V), T3
            X = np.linalg.solve(K.T @ K + eye, (M @ K).T).T
            if mode == 0:
                U = X
            elif mode == 1:
                V = X
            else:
                W = X
    sv = np.abs(V).max(0)
    sw = np.abs(W).max(0)
    return U * (sv * sw), V / sv, W / sw


def _host_constants(W, b, trans):
    W = W.astype(np.float64)
    b = b.astype(np.float64)
    U, Vf, Wf = _cp_fit(trans)

    # emission weights, 64 columns: 0..19 normalized (col0 = W_0, col i =
    # W_i - W_0), 32..51 raw (W_i); scaled into fp8 range by esc (pow2)
    Wn = np.zeros((D, 64))
    Wn[:, 0] = W[:, 0]
    Wn[:, 1:LC] = W[:, 1:] - W[:, 0:1]
    Wn[:, EROW:EROW + LC] = W
    esc = float(2.0 ** np.floor(np.log2(235.0 / np.abs(Wn).max())))
    wq = np.clip(Wn * esc, -240, 240).astype(NP_FP8)
    # [p, P, ko, m]: row (P*256 + ko*128 + p) -> w5[p, P, ko, m]
    w5 = np.ascontiguousarray(
        wq.reshape(2, 2, 128, 64).transpose(2, 0, 1, 3))

    escn = np.zeros((LC, 1), np.float32)    # normalized block scale
    escn[1:, 0] = 1.0 / esc
    ebin = np.zeros((LC, 1), np.float32)
    ebin[1:, 0] = b[1:] - b[0]
    escr = np.full((LC, 1), 1.0 / esc, np.float32)   # raw block scale
    ebir = b.reshape(LC, 1).astype(np.float32)

    vw = np.zeros((52, 2 * R), NP_BF16)
    vw[0:LC, 0:R] = Vf
    vw[EROW:EROW + LC, R:2 * R] = Wf
    u_sb = np.ascontiguousarray(U.T).astype(NP_BF16)        # [R, 20]

    normmat = np.zeros((LC, LC), NP_BF16)
    for i in range(1, LC):
        normmat[i, i] = 1.0
        normmat[0, i] = -1.0
    ones_row = np.ones((1, LC), np.float32)
    # z constant per tree: ln-scale shift repayment (170 FULL nodes) plus
    # b_0 for the 171 nodes (FULL + root) whose state-0 rows bypass the bias
    zcon = np.full((1, TPC), 170.0 * SHIFT + 171.0 * b[0], np.float32)
    return {
        "wemis": w5, "escn": escn, "ebin": ebin, "escr": escr, "ebir": ebir,
        "vw": vw, "umat": u_sb, "normmat": normmat, "onesr": ones_row,
        "zcon": zcon,
        "zscale": np.full((1, 1), 1.0 / esc, np.float32),
    }


def _bitrev(d):
    """Bit-reversal permutation of 2^d node indices: column position x holds
    node bitreverse(x), so left children occupy the first half of each level
    block and every deinterleave is a dense block copy."""
    n = 1 << d
    perm = np.zeros(n, np.int64)
    for x in range(n):
        v, q = x, 0
        for _ in range(d):
            q = (q << 1) | (v & 1)
            v >>= 1
        perm[x] = q
    return perm


def _host_ht(h, core):
    """fp8 [2, NDMAT, 128, 2, DMATILE] for one core: level-major columns,
    bit-reversed node order / tree-innermost inside each level, padded."""
    hk = h[core * TPC:(core + 1) * TPC]          # [8, 1023, 512]
    blocks = []
    for d in LEVEL_ORDER:
        blk = hk[:, (1 << d) - 1:(1 << (d + 1)) - 1, :]   # [t, q, D]
        blk = blk[:, _bitrev(d), :]
        blocks.append(blk.transpose(2, 1, 0).reshape(D, -1))  # col = x*8+t
    out = np.concatenate(blocks, axis=1)          # [512, 8184]
    pad = np.zeros((D, NCOLP - NCOL), np.float32)
    out = np.concatenate([out, pad], axis=1)      # [512, 8192]
    hq = np.clip(out, -240, 240).astype(NP_FP8)
    h5 = hq.reshape(2, 2, 128, NDMAT, DMATILE).transpose(0, 3, 2, 1, 4)
    return np.ascontiguousarray(h5)


def _patch_act_tables(nc):
    """Retarget every activation-table load to natural_log_exp_and_others
    (covers Exp, Ln and Identity) and drop the now-redundant reloads."""
    from concourse.hw_specs import get_activation_tables
    tables = list(get_activation_tables(nc.m.arch).items())
    target = None
    for idx, (name, _fns) in enumerate(tables):
        if name == "natural_log_exp_and_others":
            target = idx
    if target is None:
        return
    for fn in nc.m.functions:
        kept = False
        for blk in fn.blocks:
            new_insts = []
            for ins in blk.instructions:
                if isinstance(ins, mybir.InstLoadActFuncSet):
                    si = ins.sync_info
                    has_sems = si is not None and (
                        len(si.on_wait) > 0 or len(si.on_update) > 0)
                    if not kept or has_sems:
                        ins.act_func_set_id = target
                        kept = True
                        new_insts.append(ins)
                    continue
                new_insts.append(ins)
            blk.instructions[:] = new_insts


def _build_bass():
    nc = bacc.Bacc("TRN2", target_bir_lowering=False)

    ht_d = nc.declare_dram_parameter("ht", [2, NDMAT, 128, 2, DMATILE], FP8,
                                     isOutput=False)
    wemis_d = nc.declare_dram_parameter("wemis", [128, 2, 2, 64], FP8,
                                        isOutput=False)
    escn_d = nc.declare_dram_parameter("escn", [LC, 1], F32, isOutput=False)
    ebin_d = nc.declare_dram_parameter("ebin", [LC, 1], F32, isOutput=False)
    escr_d = nc.declare_dram_parameter("escr", [LC, 1], F32, isOutput=False)
    ebir_d = nc.declare_dram_parameter("ebir", [LC, 1], F32, isOutput=False)
    vw_d = nc.declare_dram_parameter("vw", [52, 2 * R], BF16, isOutput=False)
    umat_d = nc.declare_dram_parameter("umat", [R, LC], BF16, isOutput=False)
    normmat_d = nc.declare_dram_parameter("normmat", [LC, LC], BF16,
                                          isOutput=False)
    onesr_d = nc.declare_dram_parameter("onesr", [1, LC], F32, isOutput=False)
    zcon_d = nc.declare_dram_parameter("zcon", [1, TPC], F32, isOutput=False)
    zscale_d = nc.declare_dram_parameter("zscale", [1, 1], F32,
                                         isOutput=False)
    out_d = nc.declare_dram_parameter("out", [LC, TPC], F32, isOutput=True)

    EXP = mybir.ActivationFunctionType.Exp
    LN = mybir.ActivationFunctionType.Ln
    IDENT = mybir.ActivationFunctionType.Identity
    ADD = mybir.AluOpType.add
    MULT = mybir.AluOpType.mult
    DR = mybir.MatmulPerfMode.DoubleRow
    AXX = mybir.AxisListType.X

    # z slots: sw0-group [0..5] = L7a L7b L5 L3 L1 L0 (raw values * esc,
    # rescaled at the end); t0-group [6..10] = L7T0 L7T1 L5 L3 L1; [11] zcon
    NZSLOT = 12
    SW0SLOT = {(7, 0): 0, (7, 512): 1, (5, 0): 2, (3, 0): 3, (1, 0): 4,
               (0, 0): 5}
    T0SLOT = {(7, 0): 6, (7, 512): 7, (5, 0): 8, (3, 0): 9, (1, 0): 10}

    with tile.TileContext(nc) as tc:
        with (
            tc.tile_pool(name="consts", bufs=1) as consts,
            tc.tile_pool(name="sw", bufs=1) as swp,
            tc.tile_pool(name="ybufs", bufs=1) as ybp,
            tc.tile_pool(name="ht0", bufs=3) as htp0,
            tc.tile_pool(name="ht1", bufs=3) as htp1,
            tc.tile_pool(name="ebufs", bufs=1) as ebp,
            tc.tile_pool(name="vtiles", bufs=3) as vtp,
            tc.tile_pool(name="ttiles", bufs=3) as ttp,
            tc.tile_pool(name="ps_em", bufs=3, space="PSUM") as ps_emp,
            tc.tile_pool(name="ps_a", bufs=2, space="PSUM") as ps_ap,
            tc.tile_pool(name="ps_b", bufs=2, space="PSUM") as ps_bp,
            tc.tile_pool(name="ps_n", bufs=1, space="PSUM") as ps_np,
        ):
            # ---- constants on the ACT hwdge queue ----
            w_sb = consts.tile([128, 2, 2, 64], FP8)
            nc.scalar.dma_start(w_sb[:], wemis_d[:])
            escn_sb = consts.tile([LC, 1], F32)
            nc.scalar.dma_start(escn_sb[:], escn_d[:])
            ebin_sb = consts.tile([LC, 1], F32)
            nc.scalar.dma_start(ebin_sb[:], ebin_d[:])
            escr_sb = consts.tile([LC, 1], F32)
            nc.scalar.dma_start(escr_sb[:], escr_d[:])
            ebir_sb = consts.tile([LC, 1], F32)
            nc.scalar.dma_start(ebir_sb[:], ebir_d[:])
            vw_sb = consts.tile([52, 2 * R], BF16)
            nc.scalar.dma_start(vw_sb[:], vw_d[:])
            u_sb = consts.tile([R, LC], BF16)
            nc.scalar.dma_start(u_sb[:], umat_d[:])
            normmat_sb = consts.tile([LC, LC], BF16)
            nc.scalar.dma_start(normmat_sb[:], normmat_d[:])
            onesr_sb = consts.tile([1, LC], F32)
            nc.scalar.dma_start(onesr_sb[:], onesr_d[:])
            zscale_sb = consts.tile([1, 1], F32)
            nc.scalar.dma_start(zscale_sb[:], zscale_d[:])

            # sw_sb semantics per level range: FAST levels hold exp(sw_abs),
            # FULL levels + L0 hold normalized sw (row0 = 0)
            sw_sb = swp.tile([LC, NCOLI], BF16)
            zparts = swp.tile([1, NZSLOT * TPC], F32)
            nc.scalar.dma_start(
                zparts[0:1, (NZSLOT - 1) * TPC:NZSLOT * TPC], zcon_d[:])
            zfin = swp.tile([1, TPC], F32)

            # per-level deinterleaved buffers; rows 20..31 zeroed once
            ybufs = {}
            for d in range(DEPTH, 0, -1):
                yb = ybp.tile([52, TPC * (1 << d) // 2], BF16,
                              tag=f"y{d}", name=f"y{d}")
                nc.gpsimd.memset(yb[0:EROW, :], 0.0)
                ybufs[d] = yb

            ebufs = [ebp.tile([52, COLTILE], BF16, tag=f"e{i}", name=f"e{i}")
                     for i in range(3)]
            ebuf_i = [0]
            pending = []   # deferred DVE z-reduces

            def flush_z():
                for args in pending:
                    nc.vector.tensor_reduce(*args)
                pending.clear()

            def zred(slot, src_ap, tcount):
                pending.append((
                    zparts[0:1, slot * TPC:(slot + 1) * TPC],
                    src_ap.rearrange("p (q t) -> p t q", t=tcount),
                    AXX, ADD))

            # ---- phase 1: emission ----------------------------------------
            INTERNAL = [(8, 2048), (7, 1024), (6, 512), (5, 256), (4, 128),
                        (3, 64), (2, 32), (1, 16), (0, 16)]  # L0 incl pad

            def emission(k):
                dt, sub = k // 4, k % 4
                if sub == 0:
                    htts = []
                    for P in range(2):
                        pool = htp0 if P == 0 else htp1
                        htt = pool.tile([128, 2, DMATILE], FP8,
                                        tag=f"htt{P}", name=f"htt{P}")
                        if dt == 0:
                            h2 = DMATILE // 2
                            nc.sync.dma_start(htt[:, :, 0:h2],
                                              ht_d[P, dt, :, :, 0:h2])
                            nc.sync.dma_start(htt[:, :, h2:],
                                              ht_d[P, dt, :, :, h2:])
                        else:
                            nc.sync.dma_start(htt[:], ht_d[P, dt])
                        htts.append(htt)
                    emission.htts = htts
                htts = emission.htts
                c0 = k * COLTILE
                ps = ps_emp.tile([64, COLTILE], F32, tag="ps_em",
                                 name="ps_em")
                for P in range(2):
                    nc.tensor.matmul(
                        ps[:], w_sb[:, P, :, :],
                        htts[P][:, :, sub * COLTILE:(sub + 1) * COLTILE],
                        start=(P == 0), stop=(P == 1), perf_mode=DR)
                if c0 < NLEAFC:
                    # leaves: raw-exp; bit-reversed order makes this a dense
                    # block write (left children first half, right second)
                    y9 = ybufs[DEPTH]
                    row = 0 if k < 4 else EROW
                    dst = c0 if k < 4 else c0 - NLEAFC // 2
                    nc.scalar.activation(
                        y9[row:row + LC, dst:dst + COLTILE],
                        ps[EROW:EROW + LC, :], EXP,
                        bias=ebir_sb[:], scale=escr_sb[:])
                    return
                # internal: split by level ranges
                ic0 = c0 - NLEAFC
                for lvl, ncols in INTERNAL:
                    lo, hi = OFFSI[lvl], OFFSI[lvl] + ncols
                    s = max(lo, ic0)
                    e = min(hi, ic0 + COLTILE)
                    if s >= e:
                        continue
                    po, w = s - ic0, e - s
                    if lvl in FAST:
                        nc.scalar.activation(
                            sw_sb[:, s:e], ps[EROW:EROW + LC, po:po + w],
                            EXP, bias=ebir_sb[:], scale=escr_sb[:])
                    else:
                        nc.scalar.activation(
                            sw_sb[:, s:e], ps[0:LC, po:po + w],
                            IDENT, bias=ebin_sb[:], scale=escn_sb[:])
                        key = (lvl, s - lo)
                        if key in SW0SLOT:
                            wz = w - (8 if lvl == 0 else 0)  # skip pad cols
                            zred(SW0SLOT[key], ps[0:1, po:po + wz], TPC)

            # ---- phase 2 tiles --------------------------------------------
            def chain_core(rhs_ap, nt, cast_dve):
                """mm1 -> cast -> mult -> mm2; returns the S psum tile."""
                psa = ps_ap.tile([2 * R, COLTILE], F32, tag="ps_a",
                                 name="ps_a")
                nc.tensor.matmul(psa[:, :nt], vw_sb[:], rhs_ap,
                                 start=True, stop=True)
                wb = vtp.tile([R, COLTILE], BF16, tag="w", name="w")
                if cast_dve:
                    nc.vector.tensor_copy(wb[:, :nt], psa[R:2 * R, :nt])
                elif nt >= 512:
                    # split the cast across ACT and DVE so the stage halves
                    h = nt // 2
                    nc.scalar.activation(wb[:, :h], psa[R:2 * R, :h], IDENT)
                    nc.vector.tensor_copy(wb[:, h:nt], psa[R:2 * R, h:nt])
                else:
                    nc.scalar.activation(wb[:, :nt], psa[R:2 * R, :nt],
                                         IDENT)
                vb = vtp.tile([R, COLTILE], BF16, tag="v", name="v")
                nc.vector.tensor_tensor(vb[:, :nt], psa[0:R, :nt],
                                        wb[:, :nt], MULT)
                flush_z()
                psb = ps_bp.tile([LC, COLTILE], F32, tag="ps_b", name="ps_b")
                nc.tensor.matmul(psb[:, :nt], u_sb[:], vb[:, :nt],
                                 start=True, stop=True)
                return psb

            def _half_segs(d, c0, nt):
                """Split tile [c0, c0+nt) at the level's half boundary:
                yields (src_off, width, yb_row, yb_col)."""
                halfc = TPC * (1 << d) // 2
                segs = []
                if c0 < halfc:
                    w = min(nt, halfc - c0)
                    segs.append((0, w, 0, c0))
                if c0 + nt > halfc:
                    s = max(c0, halfc)
                    segs.append((s - c0, c0 + nt - s, EROW, s - halfc))
                return segs

            def fast_tile(d, c0, nt):
                yprev = ybufs[d + 1]
                if d == DEPTH - 1:
                    rhs = yprev[:, c0:c0 + nt]      # leaves already exp
                else:
                    eb = ebufs[ebuf_i[0] % 3]
                    ebuf_i[0] += 1
                    nc.scalar.activation(eb[:, :nt], yprev[:, c0:c0 + nt],
                                         EXP)
                    rhs = eb[:, :nt]
                psb = chain_core(rhs, nt, cast_dve=(d == 8))
                # F = exp(sw_abs) * S: dense block copies per half
                p_off = OFFSI[d]
                yb = ybufs[d]
                for so, w, row, dst in _half_segs(d, c0, nt):
                    nc.vector.tensor_tensor(
                        yb[row:row + LC, dst:dst + w],
                        psb[0:LC, so:so + w],
                        sw_sb[:, p_off + c0 + so:p_off + c0 + so + w], MULT)

            def full_tile(d, c0, nt):
                yprev = ybufs[d + 1]
                psb = chain_core(yprev[:, c0:c0 + nt], nt, cast_dve=False)
                tb = ttp.tile([LC, COLTILE], BF16, tag="t", name="t")
                nc.scalar.activation(tb[:, :nt], psb[:, :nt], LN, scale=SLN)
                psn = ps_np.tile([LC, COLTILE], F32, tag="ps_n", name="ps_n")
                nc.tensor.matmul(psn[:, :nt], normmat_sb[:], tb[:, :nt],
                                 start=True, stop=True)
                p_off = OFFSI[d]
                yb = ybufs[d]
                for so, w, row, dst in _half_segs(d, c0, nt):
                    nc.vector.tensor_add(
                        yb[row:row + LC, dst:dst + w],
                        psn[0:LC, so:so + w],
                        sw_sb[:, p_off + c0 + so:p_off + c0 + so + w])
                zred(T0SLOT[(d, c0)], tb[0:1, :nt], TPC)

            def full_pair(d):
                """Two independent same-level tiles, ops stage-interleaved so
                each engine FIFO alternates A/B and B rides in A's waits."""
                nt = COLTILE
                yprev = ybufs[d + 1]
                psas, wbs, vbs, psbs, tbs, psns = [], [], [], [], [], []
                for c0 in (0, COLTILE):
                    psa = ps_ap.tile([2 * R, COLTILE], F32, tag="ps_a",
                                     name="ps_a")
                    nc.tensor.matmul(psa[:], vw_sb[:],
                                     yprev[:, c0:c0 + nt],
                                     start=True, stop=True)
                    psas.append(psa)
                for i in range(2):
                    wb = vtp.tile([R, COLTILE], BF16, tag="w", name="w")
                    nc.scalar.activation(wb[:, 0:COLTILE // 2],
                                         psas[i][R:2 * R, 0:COLTILE // 2],
                                         IDENT)
                    nc.vector.tensor_copy(wb[:, COLTILE // 2:],
                                          psas[i][R:2 * R, COLTILE // 2:])
                    wbs.append(wb)
                flush_z()
                for i in range(2):
                    vb = vtp.tile([R, COLTILE], BF16, tag="v", name="v")
                    nc.vector.tensor_tensor(vb[:], psas[i][0:R, :],
                                            wbs[i][:], MULT)
                    vbs.append(vb)
                for i in range(2):
                    psb = ps_bp.tile([LC, COLTILE], F32, tag="ps_b",
                                     name="ps_b")
                    nc.tensor.matmul(psb[:], u_sb[:], vbs[i][:],
                                     start=True, stop=True)
                    psbs.append(psb)
                for i in range(2):
                    tb = ttp.tile([LC, COLTILE], BF16, tag="t", name="t")
                    nc.scalar.activation(tb[:], psbs[i][:], LN, scale=SLN)
                    tbs.append(tb)
                for i in range(2):
                    if i == 0:
                        # reuse a retired mm1 psum bank (same byte footprint)
                        pst = ps_ap.tile([2 * R, COLTILE], F32, tag="ps_a",
                                         name="ps_n")
                    else:
                        pst = ps_np.tile([LC, COLTILE], F32, tag="ps_n",
                                         name="ps_n")
                    nc.tensor.matmul(pst[0:LC, :], normmat_sb[:], tbs[i][:],
                                     start=True, stop=True)
                    psns.append(pst)
                p_off = OFFSI[d]
                yb = ybufs[d]
                for i, c0 in enumerate((0, COLTILE)):
                    for so, w, row, dst in _half_segs(d, c0, nt):
                        nc.vector.tensor_add(
                            yb[row:row + LC, dst:dst + w],
                            psns[i][0:LC, so:so + w],
                            sw_sb[:, p_off + c0 + so:p_off + c0 + so + w])
                for i, c0 in enumerate((0, COLTILE)):
                    zred(T0SLOT[(d, c0)], tbs[i][0:1, :], TPC)

            # program order: FAST-8 tile i only needs leaf tiles i/4+i and
            # esw tile 8+i, and each deeper level only its own sw tile, so
            # the serial tail starts well inside the DMA window
            for k in range(8):
                emission(k)
            for i in range(4):
                emission(8 + i)
                fast_tile(8, i * COLTILE, COLTILE)
            emission(12)
            emission(13)
            full_pair(7)
            emission(14)
            fast_tile(6, 0, COLTILE)
            emission(15)
            full_tile(5, 0, 256)
            fast_tile(4, 0, 128)
            full_tile(3, 0, 64)
            fast_tile(2, 0, 32)
            full_tile(1, 0, 16)

            # ---- root + finale --------------------------------------------
            # z assembly is emitted inside the root chain so the reduces and
            # the broadcast matmul overlap the root's own ops
            eb = ebufs[ebuf_i[0] % 3]
            ebuf_i[0] += 1
            nc.scalar.activation(eb[:, :TPC], ybufs[1][:, 0:TPC], EXP)
            psb_root = chain_core(eb[:, :TPC], TPC, cast_dve=False)

            zS = swp.tile([1, TPC], F32)
            nc.vector.tensor_reduce(
                zS[:], zparts[0:1, 0:6 * TPC].rearrange(
                    "p (q t) -> p t q", t=TPC), AXX, ADD)
            zT = swp.tile([1, TPC], F32)
            nc.vector.tensor_reduce(
                zT[:], zparts[0:1, 6 * TPC:].rearrange(
                    "p (q t) -> p t q", t=TPC), AXX, ADD)
            nc.vector.scalar_tensor_tensor(
                zfin[:], zS[:], zscale_sb[:], zT[:], MULT, ADD)
            qps = ps_np.tile([LC, COLTILE], F32, tag="ps_n", name="ps_n")
            nc.tensor.matmul(qps[:, :TPC], onesr_sb[:], zfin[:],
                             start=True, stop=True)

            troot = ttp.tile([LC, COLTILE], F32, tag="troot", name="troot")
            nc.scalar.activation(troot[:, :TPC], psb_root[:, :TPC], LN)
            o1 = swp.tile([LC, TPC], F32)
            nc.vector.tensor_add(o1[:], troot[:, :TPC],
                                 sw_sb[:, OFFSI[0]:OFFSI[0] + TPC])
            o2 = swp.tile([LC, TPC], F32)
            nc.vector.tensor_add(o2[:], o1[:], qps[:, :TPC])
            nc.sync.dma_start(out_d[:], o2[:])

    nc.compile()
    _patch_act_tables(nc)
    return nc


_CACHE = {}


def _get_nc():
    if "nc" not in _CACHE:
        _CACHE["nc"] = _build_bass()
    return _CACHE["nc"]


def run(h, W, b, trans, trace=False, **trace_kwargs):
    h = np.asarray(h, dtype=np.float32)
    W = np.asarray(W, dtype=np.float32)
    b = np.asarray(b, dtype=np.float32)
    trans = np.asarray(trans, dtype=np.float32)

    consts = _host_constants(W, b, trans)
    in_maps = []
    for core in range(NCORES):
        m = dict(consts)
        m["ht"] = _host_ht(h, core)
        in_maps.append(m)

    nc = _get_nc()
    res = run_bass_kernel_spmd(nc, in_maps, list(range(NCORES)),
                               trace=trace, **trace_kwargs)
    outs = [res.results[k]["out"] for k in range(NCORES)]  # each [20, 8]
    full = np.concatenate([np.asarray(o, np.float32).T for o in outs],
                          axis=0).reshape(B, L, C)
    return np.ascontiguousarray(full), res


def kernel(h, W, b, trans):
    out, _ = run(h, W, b, trans, trace=False)
    return out
